# revision 17
# baseline (speedup 1.0000x reference)
"""Causal self-attention (B=4, T=2048, D=1024, H=16) on 8 trn2 NeuronCores.

Sharding: tensor-parallel over heads — 2 heads per core. Each core computes
qkv projections for its 2 heads (from replicated x), causal attention, and a
partial output projection (its 128 rows of w_proj). Host sums the 8 partial
[S, D] outputs.

Per-core kernel formulation (everything bf16 into the PE, fp32 PSUM accum):
  xT [D, S] (host-pretransposed)  ->  qT, kT = w.T @ xT  [128, S]
  vT = wv.T @ xT, then PE-transposed into v blocks [S, 128]
  scores (transposed): sT[j, i] = kT-as-lhsT @ qT-as-rhs, per (b, head),
    2 heads packed in PE row-groups (K=64 each)
  p = exp(sT / 8) (ScalarE, causal-sliced), straddle diagonal masked by a
    lower-tri multiply
  oT[d, i] = [v_h | ones].T @ p accumulated over j tiles; the ones columns
    produce the softmax denominator rows for free
  normalize: reciprocal of gathered denominators, broadcast over 64
    partitions via a tiny K=2 matmul, multiply
  out partial = oT-as-lhsT @ w_proj-rows-as-rhs  [S, D] fp32 -> HBM
"""

import math

import numpy as np
import ml_dtypes

B, T, D, H = 4, 2048, 1024, 16
HD = D // H           # 64
S = B * T             # 8192
P = 128
KT = D // P           # 8 k-tiles over D
MC = S // 512         # 16 m-chunks of 512
NT = S // P           # 64 m-tiles of 128
JT = T // P           # 16 j-tiles per batch
NCH = T // 512        # 4 i-chunks per batch
N_CORES = 8

BFNP = ml_dtypes.bfloat16

_CACHE = {}


def _build_nc():
    import concourse.tile as tile
    import concourse.mybir as mybir
    from concourse import bacc

    BF = mybir.dt.bfloat16
    F32 = mybir.dt.float32
    Exp = mybir.ActivationFunctionType.Exp

    nc = bacc.Bacc("TRN2", num_devices=N_CORES)

    xT = nc.dram_tensor("xT", [D, S], BF, kind="ExternalInput").ap()
    wq = nc.dram_tensor("wq", [D, P], BF, kind="ExternalInput").ap()
    wk = nc.dram_tensor("wk", [D, P], BF, kind="ExternalInput").ap()
    wv = nc.dram_tensor("wv", [D, P], BF, kind="ExternalInput").ap()
    wp = nc.dram_tensor("wp", [P, D], BF, kind="ExternalInput").ap()
    maskt = nc.dram_tensor("maskt", [P, P], BF, kind="ExternalInput").ap()
    ident = nc.dram_tensor("ident", [P, P], BF, kind="ExternalInput").ap()
    e01p = nc.dram_tensor("e01p", [P, P], F32, kind="ExternalInput").ap()
    out_p = nc.dram_tensor("out_p", [S, D], BF, kind="ExternalOutput").ap()

    with tile.TileContext(nc) as tc:
        with tc.tile_pool(name="singles", bufs=1) as singles:
            qT_sb = singles.tile([P, S], BF)
            kT_sb = singles.tile([P, S], BF)
            oT_sb = singles.tile([P, S], BF)
            # v blocks per m-tile: [v_h0 | ones | v_h1 | ones] (65-wide lhsTs)
            v_sb = singles.tile([P, NT, 130], BF)
            wq_sb = singles.tile([P, KT, P], BF)
            wk_sb = singles.tile([P, KT, P], BF)
            wv_sb = singles.tile([P, KT, P], BF)
            wp_sb = singles.tile([P, D], BF)
            mask_sb = singles.tile([P, P], BF)
            id_sb = singles.tile([P, P], BF)
            vT_sb = singles.tile([P, S], BF)
            e01p_sb = singles.tile([P, P], F32)
            r2_sb = singles.tile([P, 512], F32)

            nc.sync.dma_start(out=wq_sb, in_=wq.rearrange("(kt p) n -> p kt n", p=P))
            nc.sync.dma_start(out=wk_sb, in_=wk.rearrange("(kt p) n -> p kt n", p=P))
            nc.sync.dma_start(out=wv_sb, in_=wv.rearrange("(kt p) n -> p kt n", p=P))
            nc.sync.dma_start(out=wp_sb, in_=wp)
            nc.sync.dma_start(out=mask_sb, in_=maskt)
            nc.sync.dma_start(out=id_sb, in_=ident)
            nc.sync.dma_start(out=e01p_sb, in_=e01p)
            nc.vector.memset(r2_sb, 0.0)
            nc.vector.memset(v_sb[:, :, 64:65], 1.0)
            nc.vector.memset(v_sb[:, :, 129:130], 1.0)

            xT_r = xT.rearrange("(kt p) m -> p kt m", p=P)

            # -------- Phase 2+3: attention with interleaved normalize+proj ------
            # PSUM budget (8 banks): s_0/s_1 bufs=2 (4) + av_0/av_1 bufs=1 (2)
            # + shared pjbc tag bufs=2 (2).
            with (
                tc.tile_pool(name="xc_pool", bufs=3) as xpool,
                tc.tile_pool(name="p_pool", bufs=6) as ppool,
                tc.tile_pool(name="dst_pool", bufs=4) as dstp,
                tc.tile_pool(name="g_pool", bufs=2) as gpool,
                tc.tile_pool(name="st_pool", bufs=4) as stp,
                tc.tile_pool(name="out_pool", bufs=4) as outp,
                tc.tile_pool(name="ps_s", bufs=2, space="PSUM") as ps2,
                tc.tile_pool(name="ps_av", bufs=1, space="PSUM") as avp,
                tc.tile_pool(name="ps_pj", bufs=2, space="PSUM") as ps3,
            ):

                def attention_b(b):
                    for c in range(NCH):
                        av_t = [avp.tile([P, 512], F32, name=f"av_{h}")
                                for h in (0, 1)]
                        pending_av = []  # software pipeline: AV one jt behind

                        def flush_av():
                            for args in pending_av:
                                nc.tensor.matmul(*args[0], **args[1])
                            pending_av.clear()

                        for jt in range(4 * c + 4):
                            diag = (jt // 4 == c)
                            off = jt * P - c * 512 if diag else 0
                            # both heads' scores in one 2-bank psum tile
                            s_ps = ps2.tile([P, 1024], F32, name="s_ps")
                            for h in (0, 1):
                                lk = kT_sb[h * 64:(h + 1) * 64,
                                           b * T + jt * P: b * T + (jt + 1) * P]
                                rq = qT_sb[h * 64:(h + 1) * 64,
                                           b * T + c * 512: b * T + (c + 1) * 512]
                                nc.tensor.matmul(s_ps[:, 512 * h: 512 * (h + 1)],
                                                 lhsT=lk, rhs=rq,
                                                 start=True, stop=True)
                            flush_av()
                            p_sb = ppool.tile([P, 1024], BF, name="p_sb")
                            scale = 1.0 / math.sqrt(HD)
                            if off < 172:
                                # single exp over both heads (junk span between
                                # the halves is never read downstream)
                                nc.scalar.activation(
                                    out=p_sb[:, off:1024], in_=s_ps[:, off:1024],
                                    func=Exp, scale=scale)
                            else:
                                for h in (0, 1):
                                    nc.scalar.activation(
                                        out=p_sb[:, 512 * h + off: 512 * (h + 1)],
                                        in_=s_ps[:, 512 * h + off: 512 * (h + 1)],
                                        func=Exp, scale=scale)
                            if diag:
                                for h in (0, 1):
                                    nc.gpsimd.tensor_mul(
                                        out=p_sb[:, 512 * h + off: 512 * h + off + P],
                                        in0=p_sb[:, 512 * h + off: 512 * h + off + P],
                                        in1=mask_sb)
                            for h in (0, 1):
                                lv = v_sb[:, b * JT + jt, 65 * h: 65 * h + 65]
                                pending_av.append((
                                    (av_t[h][0:65, off:512],),
                                    dict(lhsT=lv,
                                         rhs=p_sb[:, 512 * h + off: 512 * (h + 1)],
                                         start=(jt == 0), stop=(jt == 4 * c + 3)),
                                ))
                        flush_av()
                        # oT (unnormalized) + denominator gather; normalize
                        # per-chunk so the reciprocal/broadcast chain hides
                        # behind the next chunk's attention
                        for h in (0, 1):
                            avt = av_t[h]
                            nc.vector.tensor_copy(
                                out=oT_sb[h * 64:(h + 1) * 64,
                                          b * T + c * 512: b * T + (c + 1) * 512],
                                in_=avt[0:64])
                            # engine APs need 32-aligned partition base: stage
                            # at partition 0, reciprocal there, DMA into row h
                            # of the broadcast-matmul rhs
                            dstage = dstp.tile([1, 512], F32, name="dstage")
                            nc.vector.tensor_copy(out=dstage, in_=avt[64:65])
                            rstage = dstp.tile([1, 512], F32, name="rstage")
                            nc.vector.reciprocal_approx_fast(out=rstage,
                                                             in_=dstage)
                            nc.sync.dma_start(out=r2_sb[h:h + 1, :], in_=rstage)
                        # broadcast rows 0/1 across the two 64-partition head
                        # groups with a K=128 indicator matmul (stays in the
                        # 128-row tiling mode; rows 2-127 of r2_sb are zero)
                        bc_ps = ps3.tile([P, 512], F32, name="bc", tag="pjbc")
                        nc.tensor.matmul(bc_ps, lhsT=e01p_sb, rhs=r2_sb,
                                         start=True, stop=True)
                        sl = slice(b * T + c * 512, b * T + (c + 1) * 512)
                        nc.vector.tensor_mul(out=oT_sb[:, sl], in0=oT_sb[:, sl],
                                             in1=bc_ps)

                def norm_proj_b(b):
                    for c in range(NCH):
                        sl = slice(b * T + c * 512, b * T + (c + 1) * 512)
                        for i in range(4):
                            mt = (b * T + c * 512) // P + i
                            ob = outp.tile([P, D], BF, name="ob")
                            for nch in range(2):
                                pj = ps3.tile([P, 512], F32, name="pj", tag="pjbc")
                                nc.tensor.matmul(
                                    pj, lhsT=oT_sb[:, mt * P:(mt + 1) * P],
                                    rhs=wp_sb[:, nch * 512:(nch + 1) * 512],
                                    start=True, stop=True)
                                nc.vector.tensor_copy(
                                    out=ob[:, nch * 512:(nch + 1) * 512], in_=pj)
                            nc.sync.dma_start(out=out_p[mt * P:(mt + 1) * P, :],
                                              in_=ob)

                def qkv_chunk(mc):
                    sl = slice(mc * 512, (mc + 1) * 512)
                    xc = xpool.tile([P, KT, 512], BF, name="xc")
                    for kt in range(KT):
                        nc.sync.dma_start(out=xc[:, kt], in_=xT_r[:, kt, sl])
                    s_qk = ps2.tile([P, 1024], F32, name="s_qk", tag="s_ps")
                    for kt in range(KT):
                        nc.tensor.matmul(s_qk[:, 0:512], lhsT=wq_sb[:, kt],
                                         rhs=xc[:, kt],
                                         start=(kt == 0), stop=(kt == KT - 1))
                        nc.tensor.matmul(s_qk[:, 512:1024], lhsT=wk_sb[:, kt],
                                         rhs=xc[:, kt],
                                         start=(kt == 0), stop=(kt == KT - 1))
                    nc.vector.tensor_copy(out=qT_sb[:, sl], in_=s_qk[:, 0:512])
                    nc.vector.tensor_copy(out=kT_sb[:, sl], in_=s_qk[:, 512:1024])
                    ps_v = ps3.tile([P, 512], F32, name="ps_v", tag="pjbc")
                    for kt in range(KT):
                        nc.tensor.matmul(ps_v, lhsT=wv_sb[:, kt], rhs=xc[:, kt],
                                         start=(kt == 0), stop=(kt == KT - 1))
                    nc.scalar.copy(out=vT_sb[:, sl], in_=ps_v)
                    for i in range(4):
                        mt = mc * 4 + i
                        ps_t = ps3.tile([P, P], BF, name="ps_t", tag="pjbc")
                        nc.tensor.transpose(
                            ps_t, vT_sb[:, mt * P:(mt + 1) * P], id_sb)
                        nc.vector.tensor_copy(out=v_sb[:, mt, 0:64],
                                              in_=ps_t[:, 0:64])
                        nc.vector.tensor_copy(out=v_sb[:, mt, 65:129],
                                              in_=ps_t[:, 64:128])

                # interleave per-batch QKV with attention: attn(b)'s exps on
                # ScalarE overlap QKV(b+1)'s PE-heavy projection matmuls
                for mc in range(4):
                    qkv_chunk(mc)
                attention_b(0)
                for b in range(1, B):
                    for mc in range(4 * b, 4 * b + 4):
                        qkv_chunk(mc)
                    attention_b(b)
                    norm_proj_b(b - 1)
                norm_proj_b(B - 1)

    nc.compile()
    return nc


def _host_inputs(x, w_qkv, w_proj):
    x = np.asarray(x, dtype=np.float32)
    w_qkv = np.asarray(w_qkv, dtype=np.float32)
    w_proj = np.asarray(w_proj, dtype=np.float32)

    xT = np.ascontiguousarray(x.reshape(S, D).T).astype(BFNP)
    mask = np.triu(np.ones((P, P), np.float32)).astype(BFNP)  # [j, i]: 1 if j<=i
    ident = np.eye(P, dtype=np.float32).astype(BFNP)
    e01p = np.zeros((P, P), np.float32)
    e01p[0, :64] = 1.0
    e01p[1, 64:] = 1.0

    in_maps = []
    for core in range(N_CORES):
        cs = slice(core * P, (core + 1) * P)
        in_maps.append({
            "xT": xT,
            "wq": np.ascontiguousarray(w_qkv[:, core * P:(core + 1) * P]).astype(BFNP),
            "wk": np.ascontiguousarray(w_qkv[:, D + core * P: D + (core + 1) * P]).astype(BFNP),
            "wv": np.ascontiguousarray(w_qkv[:, 2 * D + core * P: 2 * D + (core + 1) * P]).astype(BFNP),
            "wp": np.ascontiguousarray(w_proj[cs, :]).astype(BFNP),
            "maskt": mask,
            "ident": ident,
            "e01p": e01p,
        })
    return in_maps


def run_spmd(x, w_qkv, w_proj, trace=False):
    """Compile (cached) + run on 8 cores. Returns (out [B,T,D] fp32, results)."""
    from concourse import bass_utils

    if "nc" not in _CACHE:
        _CACHE["nc"] = _build_nc()
    nc = _CACHE["nc"]

    in_maps = _host_inputs(x, w_qkv, w_proj)
    res = bass_utils.run_bass_kernel_spmd(
        nc, in_maps, core_ids=list(range(N_CORES)), trace=trace)

    acc = np.zeros((S, D), np.float32)
    for r in res.results:
        acc += np.asarray(r["out_p"]).astype(np.float32)
    return acc.reshape(B, T, D), res


def kernel(x, w_qkv, w_proj):
    out, _ = run_spmd(x, w_qkv, w_proj, trace=False)
    return out



# revision 18
# speedup vs baseline: 1.1882x; 1.1882x over previous
"""Causal self-attention (B=4, T=2048, D=1024, H=16) on 8 trn2 NeuronCores.

Sharding: tensor-parallel over heads — 2 heads per core. Each core computes
qkv projections for its 2 heads (from replicated x), causal attention, and a
partial output projection (its 128 rows of w_proj). Host sums the 8 partial
[S, D] outputs.

Per-core kernel formulation (everything bf16 into the PE, fp32 PSUM accum):
  xT [D, S] (host-pretransposed)  ->  qT, kT = w.T @ xT  [128, S]
  vT = wv.T @ xT, then PE-transposed into v blocks [S, 128]
  scores (transposed): sT[j, i] = kT-as-lhsT @ qT-as-rhs, per (b, head),
    2 heads packed in PE row-groups (K=64 each)
  p = exp(sT / 8) (ScalarE, causal-sliced), straddle diagonal masked by a
    lower-tri multiply
  oT[d, i] = [v_h | ones].T @ p accumulated over j tiles; the ones columns
    produce the softmax denominator rows for free
  normalize: reciprocal of gathered denominators, broadcast over 64
    partitions via a tiny K=2 matmul, multiply
  out partial = oT-as-lhsT @ w_proj-rows-as-rhs  [S, D] fp32 -> HBM
"""

import math

import numpy as np
import ml_dtypes

B, T, D, H = 4, 2048, 1024, 16
HD = D // H           # 64
S = B * T             # 8192
P = 128
KT = D // P           # 8 k-tiles over D
MC = S // 512         # 16 m-chunks of 512
NT = S // P           # 64 m-tiles of 128
JT = T // P           # 16 j-tiles per batch
NCH = T // 512        # 4 i-chunks per batch
N_CORES = 8

BFNP = ml_dtypes.bfloat16

_CACHE = {}


def _build_nc():
    import concourse.tile as tile
    import concourse.mybir as mybir
    from concourse import bacc

    BF = mybir.dt.bfloat16
    F32 = mybir.dt.float32
    Exp = mybir.ActivationFunctionType.Exp

    nc = bacc.Bacc("TRN2", num_devices=N_CORES)

    xT = nc.dram_tensor("xT", [D, S], BF, kind="ExternalInput").ap()
    wq = nc.dram_tensor("wq", [D, P], BF, kind="ExternalInput").ap()
    wk = nc.dram_tensor("wk", [D, P], BF, kind="ExternalInput").ap()
    wv = nc.dram_tensor("wv", [D, P], BF, kind="ExternalInput").ap()
    wp = nc.dram_tensor("wp", [P, D], BF, kind="ExternalInput").ap()
    maskt = nc.dram_tensor("maskt", [P, P], BF, kind="ExternalInput").ap()
    e01p = nc.dram_tensor("e01p", [P, P], BF, kind="ExternalInput").ap()
    ident = nc.dram_tensor("ident", [P, P], BF, kind="ExternalInput").ap()
    out_pT = nc.dram_tensor("out_pT", [D, S], BF, kind="ExternalOutput").ap()

    with tile.TileContext(nc) as tc:
        with tc.tile_pool(name="singles", bufs=1) as singles:
            qT_sb = singles.tile([P, S], BF)
            kT_sb = singles.tile([P, S], BF)
            oT_sb = singles.tile([P, S], BF)
            # v blocks per m-tile: [v_h0 | ones | v_h1 | ones] (65-wide lhsTs)
            v_sb = singles.tile([P, NT, 130], BF)
            wq_sb = singles.tile([P, KT, P], BF)
            wk_sb = singles.tile([P, KT, P], BF)
            wv_sb = singles.tile([P, KT, P], BF)
            wp_sb = singles.tile([P, D], BF)
            mask_sb = singles.tile([P, P], BF)
            e01p_sb = singles.tile([P, P], BF)
            r2_sb = singles.tile([P, 512], BF)
            id_sb = singles.tile([P, P], BF)
            vT_sb = singles.tile([P, S], BF)

            nc.sync.dma_start(out=wq_sb, in_=wq.rearrange("(kt p) n -> p kt n", p=P))
            nc.sync.dma_start(out=wk_sb, in_=wk.rearrange("(kt p) n -> p kt n", p=P))
            nc.sync.dma_start(out=wv_sb, in_=wv.rearrange("(kt p) n -> p kt n", p=P))
            nc.sync.dma_start(out=wp_sb, in_=wp)
            nc.sync.dma_start(out=mask_sb, in_=maskt)
            nc.sync.dma_start(out=e01p_sb, in_=e01p)
            nc.vector.memset(r2_sb, 0.0)
            nc.sync.dma_start(out=id_sb, in_=ident)
            nc.vector.memset(v_sb[:, :, 64:65], 1.0)
            nc.vector.memset(v_sb[:, :, 129:130], 1.0)

            xT_r = xT.rearrange("(kt p) m -> p kt m", p=P)

            # -------- Phase 2+3: attention with interleaved normalize+proj ------
            # PSUM budget (8 banks): s_0/s_1 bufs=2 (4) + av_0/av_1 bufs=1 (2)
            # + shared pjbc tag bufs=2 (2).
            with (
                tc.tile_pool(name="xc_pool", bufs=3) as xpool,
                tc.tile_pool(name="p_pool", bufs=6) as ppool,
                tc.tile_pool(name="dst_pool", bufs=4) as dstp,
                tc.tile_pool(name="g_pool", bufs=2) as gpool,
                tc.tile_pool(name="st_pool", bufs=4) as stp,
                tc.tile_pool(name="out_pool", bufs=4) as outp,
                tc.tile_pool(name="ps_s", bufs=2, space="PSUM") as ps2,
                tc.tile_pool(name="ps_av", bufs=1, space="PSUM") as avp,
                tc.tile_pool(name="ps_pj", bufs=2, space="PSUM") as ps3,
            ):
                g_tiles = {}

                def attention_b(b, interleave=None):
                    g_b = gpool.tile([8, 512], F32, name="g_b")
                    g_tiles[b] = g_b
                    for c in range(NCH):
                        if interleave is not None:
                            interleave(c)
                        av_t = [avp.tile([P, 512], F32, name=f"av_{h}")
                                for h in (0, 1)]
                        pending_av = []  # software pipeline: AV one jt behind

                        def flush_av():
                            for args in pending_av:
                                nc.tensor.matmul(*args[0], **args[1])
                            pending_av.clear()

                        for jt in range(4 * c + 4):
                            diag = (jt // 4 == c)
                            off = jt * P - c * 512 if diag else 0
                            # both heads' scores in one 2-bank psum tile
                            s_ps = ps2.tile([P, 1024], F32, name="s_ps")
                            for h in (0, 1):
                                lk = kT_sb[h * 64:(h + 1) * 64,
                                           b * T + jt * P: b * T + (jt + 1) * P]
                                rq = qT_sb[h * 64:(h + 1) * 64,
                                           b * T + c * 512: b * T + (c + 1) * 512]
                                nc.tensor.matmul(s_ps[:, 512 * h: 512 * (h + 1)],
                                                 lhsT=lk, rhs=rq,
                                                 start=True, stop=True)
                            flush_av()
                            p_sb = ppool.tile([P, 1024], BF, name="p_sb")
                            scale = 1.0 / math.sqrt(HD)
                            if off < 172:
                                # single exp over both heads (junk span between
                                # the halves is never read downstream)
                                nc.scalar.activation(
                                    out=p_sb[:, off:1024], in_=s_ps[:, off:1024],
                                    func=Exp, scale=scale)
                            else:
                                for h in (0, 1):
                                    nc.scalar.activation(
                                        out=p_sb[:, 512 * h + off: 512 * (h + 1)],
                                        in_=s_ps[:, 512 * h + off: 512 * (h + 1)],
                                        func=Exp, scale=scale)
                            if diag:
                                for h in (0, 1):
                                    nc.gpsimd.tensor_mul(
                                        out=p_sb[:, 512 * h + off: 512 * h + off + P],
                                        in0=p_sb[:, 512 * h + off: 512 * h + off + P],
                                        in1=mask_sb)
                            for h in (0, 1):
                                lv = v_sb[:, b * JT + jt, 65 * h: 65 * h + 65]
                                pending_av.append((
                                    (av_t[h][0:65, off:512],),
                                    dict(lhsT=lv,
                                         rhs=p_sb[:, 512 * h + off: 512 * (h + 1)],
                                         start=(jt == 0), stop=(jt == 4 * c + 3)),
                                ))
                        flush_av()
                        # oT (unnormalized) + denominator gather
                        for h in (0, 1):
                            avt = av_t[h]
                            nc.vector.tensor_copy(
                                out=oT_sb[h * 64:(h + 1) * 64,
                                          b * T + c * 512: b * T + (c + 1) * 512],
                                in_=avt[0:64])
                            # engine APs need 32-aligned partition base; stage
                            # at partition 0 then DMA-scatter into g_b
                            dstage = dstp.tile([1, 512], F32, name="dstage")
                            nc.vector.tensor_copy(out=dstage, in_=avt[64:65])
                            nc.sync.dma_start(out=g_b[c * 2 + h: c * 2 + h + 1, :],
                                              in_=dstage)
                    r_b = gpool.tile([8, 512], F32, name="r_b")
                    nc.vector.reciprocal_approx_fast(out=r_b, in_=g_b)
                    r_bf = gpool.tile([8, 512], BF, name="r_bf")
                    nc.vector.tensor_copy(out=r_bf, in_=r_b)
                    g_tiles[b] = r_bf

                def norm_proj_b(b):
                    r_b = g_tiles[b]
                    for c in range(NCH):
                        # rows 0/1 of r2_sb carry this chunk's reciprocal
                        # denominators; rows 2-127 stay zero so the K=128
                        # indicator matmul (128-row mode, no mode switch)
                        # broadcasts row h over head h's 64 partitions
                        nc.sync.dma_start(out=r2_sb[0:2, :],
                                          in_=r_b[c * 2: c * 2 + 2, :])
                        bc_ps = ps3.tile([P, 512], F32, name="bc", tag="pjbc")
                        nc.tensor.matmul(bc_ps, lhsT=e01p_sb, rhs=r2_sb,
                                         start=True, stop=True)
                        sl = slice(b * T + c * 512, b * T + (c + 1) * 512)
                        nc.vector.tensor_mul(out=oT_sb[:, sl], in0=oT_sb[:, sl],
                                             in1=bc_ps)
                    # transposed projection: wp block stays stationary across
                    # the batch's 4 chunks (LDWEIGHTS amortized 4x); host
                    # transposes the [D, S] partial back
                    for nb in range(8):
                        for c in range(NCH):
                            sl = slice(b * T + c * 512, b * T + (c + 1) * 512)
                            pj = ps3.tile([P, 512], F32, name="pj", tag="pjbc")
                            nc.tensor.matmul(
                                pj, lhsT=wp_sb[:, nb * P:(nb + 1) * P],
                                rhs=oT_sb[:, sl], start=True, stop=True)
                            obT = outp.tile([P, 512], BF, name="obT")
                            nc.vector.tensor_copy(out=obT, in_=pj)
                            nc.sync.dma_start(
                                out=out_pT[nb * P:(nb + 1) * P, sl], in_=obT)

                def qkv_chunk(mc):
                    sl = slice(mc * 512, (mc + 1) * 512)
                    xc = xpool.tile([P, KT, 512], BF, name="xc")
                    for kt in range(KT):
                        nc.sync.dma_start(out=xc[:, kt], in_=xT_r[:, kt, sl])
                    s_qk = ps2.tile([P, 1024], F32, name="s_qk", tag="s_ps")
                    for kt in range(KT):
                        nc.tensor.matmul(s_qk[:, 0:512], lhsT=wq_sb[:, kt],
                                         rhs=xc[:, kt],
                                         start=(kt == 0), stop=(kt == KT - 1))
                        nc.tensor.matmul(s_qk[:, 512:1024], lhsT=wk_sb[:, kt],
                                         rhs=xc[:, kt],
                                         start=(kt == 0), stop=(kt == KT - 1))
                    nc.vector.tensor_copy(out=qT_sb[:, sl], in_=s_qk[:, 0:512])
                    nc.vector.tensor_copy(out=kT_sb[:, sl], in_=s_qk[:, 512:1024])
                    ps_v = ps3.tile([P, 512], F32, name="ps_v", tag="pjbc")
                    for kt in range(KT):
                        nc.tensor.matmul(ps_v, lhsT=wv_sb[:, kt], rhs=xc[:, kt],
                                         start=(kt == 0), stop=(kt == KT - 1))
                    nc.scalar.copy(out=vT_sb[:, sl], in_=ps_v)
                    for i in range(4):
                        mt = mc * 4 + i
                        ps_t = ps3.tile([P, P], BF, name="ps_t", tag="pjbc")
                        nc.tensor.transpose(
                            ps_t, vT_sb[:, mt * P:(mt + 1) * P], id_sb)
                        nc.vector.tensor_copy(out=v_sb[:, mt, 0:64],
                                              in_=ps_t[:, 0:64])
                        nc.vector.tensor_copy(out=v_sb[:, mt, 65:129],
                                              in_=ps_t[:, 64:128])

                # interleave per-batch QKV with attention: attn(b)'s exps on
                # ScalarE overlap QKV(b+1)'s PE-heavy projection matmuls
                for mc in range(4):
                    qkv_chunk(mc)
                attention_b(0)
                for b in range(1, B):
                    for mc in range(4 * b, 4 * b + 4):
                        qkv_chunk(mc)
                    attention_b(b)
                    norm_proj_b(b - 1)
                norm_proj_b(B - 1)

    nc.compile()
    return nc


def _host_inputs(x, w_qkv, w_proj):
    x = np.asarray(x, dtype=np.float32)
    w_qkv = np.asarray(w_qkv, dtype=np.float32)
    w_proj = np.asarray(w_proj, dtype=np.float32)

    xT = np.ascontiguousarray(x.reshape(S, D).T).astype(BFNP)
    mask = np.triu(np.ones((P, P), np.float32)).astype(BFNP)  # [j, i]: 1 if j<=i
    e01p = np.zeros((P, P), np.float32)
    e01p[0, :64] = 1.0
    e01p[1, 64:] = 1.0
    e01p = e01p.astype(BFNP)
    ident = np.eye(P, dtype=np.float32).astype(BFNP)

    in_maps = []
    for core in range(N_CORES):
        cs = slice(core * P, (core + 1) * P)
        in_maps.append({
            "xT": xT,
            "wq": np.ascontiguousarray(w_qkv[:, core * P:(core + 1) * P]).astype(BFNP),
            "wk": np.ascontiguousarray(w_qkv[:, D + core * P: D + (core + 1) * P]).astype(BFNP),
            "wv": np.ascontiguousarray(w_qkv[:, 2 * D + core * P: 2 * D + (core + 1) * P]).astype(BFNP),
            "wp": np.ascontiguousarray(w_proj[cs, :]).astype(BFNP),
            "maskt": mask,
            "e01p": e01p,
            "ident": ident,
        })
    return in_maps


def run_spmd(x, w_qkv, w_proj, trace=False):
    """Compile (cached) + run on 8 cores. Returns (out [B,T,D] fp32, results)."""
    from concourse import bass_utils

    if "nc" not in _CACHE:
        _CACHE["nc"] = _build_nc()
    nc = _CACHE["nc"]

    in_maps = _host_inputs(x, w_qkv, w_proj)
    res = bass_utils.run_bass_kernel_spmd(
        nc, in_maps, core_ids=list(range(N_CORES)), trace=trace)

    acc = np.zeros((D, S), np.float32)
    for r in res.results:
        acc += np.asarray(r["out_pT"]).astype(np.float32)
    return acc.T.reshape(B, T, D), res


def kernel(x, w_qkv, w_proj):
    out, _ = run_spmd(x, w_qkv, w_proj, trace=False)
    return out

